# revision 1
# baseline (speedup 1.0000x reference)
# Trainium2 Bass kernel for nn_BertAdapter_SLT_49933289783411
#
# Reference computation:
#   y   = tt_linear(x) + bias          (TT-factorized 768->768 linear)
#   out = x + gelu_exact(y)
#
# Key math: the TT cores with ranks [1,5,5,5,5,5,1] factor the 768x768
# weight as W = A @ B with A:(768,5), B:(5,768).  We precompute A,B on
# host (tiny, exact) and run a rank-5 bottleneck matmul on device.
#
# Sharding: data-parallel over the batch dim (8 batch elements -> 8 cores).
# Each core handles x_c:(512,768), pre-transposed on host to x^T (feature-
# major) so the contraction dim lands on SBUF partitions.  Per core:
#   t3    = A^T @ x^T              (5,512)   PSUM accumulate over f-chunks
#   y^T_j = B_j^T @ t3_pad         (128,512) per 128-feature output chunk j
#   o^T_j = x^T_j + gelu_exact(y^T_j + bias_j)
# The host transposes the gathered o^T back.
#
# Structure for latency hiding: the 512 batch rows are processed as two
# halves.  Half 0's entire output pipeline (mm2 -> gelu -> residual ->
# store) runs while half 1's x is still streaming from HBM, hiding the
# ~3us DMA completion-semaphore latency and overlapping store with load
# traffic.  mm2 runs in bf16 (B and t3 rounded): the TT branch is only
# ~4% of output magnitude, so this costs ~1e-4 relative error while
# halving PE passes.  The x loads, mm1, gelu and residual stay fp32.
#
# All constants (A, bias, B zero-padded to rank 128) are packed into the
# head of the single input tensor so one HWDGE queue carries everything.

import numpy as np
import ml_dtypes

import concourse.bass as bass
import concourse.bacc as bacc
import concourse.mybir as mybir
import concourse.tile as tile
from concourse.tile import add_dep_helper
from concourse.bass_utils import run_bass_kernel_spmd

HID = 768
ROWS = 512          # rows per core (one batch element)
# row split sizes for the two pipelined parts (A/B-tested: symmetric beats
# 320/192 — a bigger first part lengthens its ACT chain, delaying part 1)
HSIZE = (256, 256)
HOFF = (0, 256)
NCORES = 8
FCH = 6             # 768 / 128 feature chunks
RANK = 5
F32 = mybir.dt.float32
BF16 = mybir.dt.bfloat16

N_WARMUP = 28       # dummy PE matmuls to trip the HAM clock un-throttle
LOAD_GROUP = 2      # x chunks per load DMA (A/B-tested vs 3)

# packed layout of the input tensor, in f32 columns:
#   [A f32 (128,30)] [bias f32 (128,6)] [B_pad bf16 (128,768) -> 384]
#   [x half0: c0..c5 x 256] [x half1: c0..c5 x 256]
A_COLS = FCH * RANK                                # 30
BIAS_COLS = FCH                                    # 6
BM_COLS = HID // 2                                 # 384
CONST_COLS = A_COLS + BIAS_COLS + BM_COLS          # 420
XT_COLS = CONST_COLS + FCH * ROWS                  # 3492

_CACHE = {}


class _LeanTileContext(tile.TileContext):
    """TileContext with a minimal exit sequence.

    The stock exit emits drain + all-engine barrier + per-sem clears +
    barrier (~2-3us).  The runtime re-initializes semaphore state on every
    NEFF execution (verified empirically: repeated executions of the same
    loaded executable stay bit-correct without the clears), so only the
    drain — which makes the kernel end wait for the output DMAs — is kept.
    """

    def _drain_and_barrier(self, tick_clock, wait_clock):
        drain_inst = self.nc.sync.drain()
        wait_clock.add_sem_waits(
            drain_inst.ins, tile.ScopedClock({None: tick_clock.global_clock})
        )
        popped = self.nc._tile_sem_poison_stack.pop()
        assert popped is self._sem_poison


def _xcol(h, c):
    return CONST_COLS + FCH * HOFF[h] + c * HSIZE[h]


def _build_program(act=None):
    if act is None:
        act = mybir.ActivationFunctionType.Gelu
    nc = bacc.Bacc(None, target_bir_lowering=False)
    xt = nc.dram_tensor("xt", [128, XT_COLS], F32, kind="ExternalInput")
    outt = nc.dram_tensor("outt", [128, FCH * ROWS], F32, kind="ExternalOutput")

    with _LeanTileContext(nc) as tc:
        with (
            tc.tile_pool(name="const", bufs=1) as cpool,
            tc.tile_pool(name="xs", bufs=1) as xpool,
            tc.tile_pool(name="work", bufs=4) as wpool,
            tc.tile_pool(name="ps_t3", bufs=1, space="PSUM") as tpool,
            tc.tile_pool(name="ps_o", bufs=4, space="PSUM") as opool,
            tc.tile_pool(name="ps_w", bufs=1, space="PSUM") as wps_pool,
        ):
            # --- PE warmup: garbage matmuls so the HAM clock gate opens
            # (keep-alive filler matmuls between the sem-gated mm1 h1 ones
            # were A/B-tested and LOST ~1.6us: their PE-FIFO occupancy
            # delays the real matmuls more than the warm clock saves)
            wsb = cpool.tile([128, 128], BF16)
            nc.gpsimd.memset(wsb[:], 0.0)
            wps = wps_pool.tile([128, 128], F32)
            for _ in range(N_WARMUP):
                nc.tensor.matmul(wps[:], wsb[:], wsb[:], start=True, stop=True)

            # t3 in bf16, zero-padded to 128 partitions so mm2 runs K=128;
            # row RANK is all-ones: paired with the bias in B_pad's row RANK
            # it folds the TT bias into mm2 (ACT then needs no bias, so gelu
            # can run on j-pairs in one op)
            t3_sb = cpool.tile([128, ROWS], BF16)
            nc.gpsimd.memset(t3_sb[:], 0.0)
            # partition writes must be 32-aligned: set rows 32..63 to one and
            # let B_pad rows 33..63 stay zero, so only row 32 (= bias) lands
            nc.gpsimd.memset(t3_sb[32:64, :], 1.0)

            x_sb = xpool.tile([128, XT_COLS], F32)
            a_view = x_sb[:, 0:A_COLS]                                   # (128,30) f32
            bias_view = x_sb[:, A_COLS : A_COLS + BIAS_COLS]             # (128,6)
            bm_view = x_sb[:, A_COLS + BIAS_COLS : CONST_COLS].bitcast(BF16)  # (128,768)

            t3_ps = [
                tpool.tile([RANK, HSIZE[h]], F32, name=f"t3_ps{h}") for h in (0, 1)
            ]

            def dma_half(h, group=LOAD_GROUP):
                # FCH/group DMAs per half; the first also carries the consts
                for d in range(FCH // group):
                    c0 = _xcol(h, group * d)
                    start = c0 if (h, d) != (0, 0) else 0
                    end = _xcol(h, group * d) + group * HSIZE[h]
                    nc.sync.dma_start(x_sb[:, start:end], xt[:, start:end])

            def mm1_half(h, after=None):
                for c in range(FCH):
                    mm = nc.tensor.matmul(
                        t3_ps[h][:],
                        a_view[:, c * RANK : (c + 1) * RANK],
                        x_sb[:, _xcol(h, c) : _xcol(h, c) + HSIZE[h]],
                        start=(c == 0),
                        stop=(c == FCH - 1),
                    )
                    if after is not None:
                        # ordering-only edge: keep these DMA-gated matmuls
                        # out of the strict PE FIFO until half 0's mm2s ran
                        add_dep_helper(
                            mm.ins, after.ins, sync=False,
                            reason="mm1 h1 after phase2 h0 matmuls",
                        )

            def phase2_half(h):
                sz, off = HSIZE[h], HOFF[h]
                nc.vector.tensor_copy(t3_sb[0:RANK, off : off + sz], t3_ps[h][:])
                first_mm = None
                for j0 in range(0, FCH, 2):
                    # two output chunks share one PSUM bank: the first matmul
                    # (start=True) clears the bank's has_written bits, the
                    # second (start=False) overwrites its still-clear region
                    o_ps = opool.tile([128, 2 * max(HSIZE)], F32, tag="o_ps")
                    for k in (0, 1):
                        mm = nc.tensor.matmul(
                            o_ps[:, k * sz : (k + 1) * sz],
                            bm_view[:, (j0 + k) * 128 : (j0 + k + 1) * 128],
                            t3_sb[:, off : off + sz],
                            start=(k == 0),
                            stop=(k == 1),
                        )
                        if first_mm is None:
                            first_mm = mm
                    # one paired gelu halves the per-op ACT overhead on the
                    # critical tail (bias already folded in via mm2)
                    g_sb = wpool.tile([128, 2 * max(HSIZE)], F32, tag="g_sb", bufs=4)
                    nc.scalar.activation(
                        g_sb[:, : 2 * sz], o_ps[:, : 2 * sz], act, scale=1.0
                    )
                    # distinct slots: an o_sb slot is only reusable after its
                    # store's completion semaphore (~2.6us receipt), so
                    # recycling would chain that latency into the pipeline
                    o_sb = wpool.tile([128, 2 * max(HSIZE)], F32, tag="o_sb", bufs=6)
                    nc.vector.tensor_add(
                        o_sb[:, : 2 * sz],
                        g_sb[:, : 2 * sz],
                        x_sb[:, _xcol(h, j0) : _xcol(h, j0) + 2 * sz],
                    )
                    # stores split across the two DGE queues so issue cost
                    # doesn't serialize behind one engine
                    for k in (0, 1):
                        dma = nc.sync if k == 0 else nc.gpsimd
                        dma.dma_start(
                            outt[:, (j0 + k) * ROWS + off : (j0 + k) * ROWS + off + sz],
                            o_sb[:, k * sz : (k + 1) * sz],
                        )
                return first_mm

            # h1's DMAs issue right behind h0's on the Sync queue, but h1's
            # PE work is emitted after phase2(0) so the strict PE FIFO
            # doesn't block half 0's output pipeline on half 1's loads.
            dma_half(0)
            mm1_half(0)
            dma_half(1)
            first_mm_h0 = phase2_half(0)
            mm1_half(1, after=first_mm_h0)
            phase2_half(1)

    nc.finalize()
    return nc


def _get_program():
    if "nc" not in _CACHE:
        _CACHE["nc"] = _build_program()
    return _CACHE["nc"]


def _host_prep(hidden_states, bias, cores):
    """Collapse TT cores to rank-5 factors; pack consts + x^T per core."""
    c0, c1, c2, c3, c4, c5 = [c.astype(np.float64) for c in cores]
    A = np.einsum("iv,vjw,wkx->ijkx", c0[0], c1, c2).reshape(HID, RANK)
    Bm = np.einsum("xpy,yqz,zr->xpqr", c3, c4, c5[:, :, 0]).reshape(RANK, HID)

    a_p = np.ascontiguousarray(
        A.reshape(FCH, 128, RANK).transpose(1, 0, 2).reshape(128, FCH * RANK)
    ).astype(np.float32)                           # (128, 30)
    bm_pad = np.zeros((128, HID), dtype=ml_dtypes.bfloat16)
    bm_pad[:RANK] = Bm.astype(ml_dtypes.bfloat16)  # (128, 768)
    # row 32 carries the TT bias; it meets the all-ones row 32 of t3_sb in mm2
    bm_pad[32] = bias.astype(ml_dtypes.bfloat16)
    bias_p = np.ascontiguousarray(bias.astype(np.float32).reshape(FCH, 128).T)

    const_block = np.concatenate(
        [a_p, bias_p, bm_pad.view(np.float32)], axis=1
    ).astype(np.float32)                           # (128, 420)

    xts = []
    for c in range(NCORES):
        xc = hidden_states[c]  # (512, 768)
        xct = xc.T.astype(np.float32)  # (768, 512)
        # per half: [p, c*sz + m~] = x^T[c*128+p, off+m~]
        blocks = [const_block]
        for h in (0, 1):
            sz, off = HSIZE[h], HOFF[h]
            blocks.append(
                xct[:, off : off + sz]
                .reshape(FCH, 128, sz)
                .transpose(1, 0, 2)
                .reshape(128, FCH * sz)
            )
        xts.append(np.ascontiguousarray(np.concatenate(blocks, axis=1)))
    return xts


def _unpack_out(outt_list):
    """outt[p, j*ROWS + m] = out[m, j*128 + p] -> (8, 512, 768)."""
    outs = []
    for outt in outt_list:
        o = outt.reshape(128, FCH, ROWS).transpose(2, 1, 0).reshape(ROWS, HID)
        outs.append(o)
    return np.stack(outs, axis=0).astype(np.float32)


def run(inputs, trace=False, **spmd_kwargs):
    hidden_states = np.asarray(inputs["hidden_states"], dtype=np.float32)
    bias = np.asarray(inputs["bias"], dtype=np.float32)
    cores = [np.asarray(inputs[f"core{i}"], dtype=np.float32) for i in range(6)]

    xts = _host_prep(hidden_states, bias, cores)
    nc = _get_program()
    in_maps = [{"xt": xts[c]} for c in range(NCORES)]
    res = run_bass_kernel_spmd(
        nc, in_maps, core_ids=list(range(NCORES)), trace=trace, **spmd_kwargs
    )
    out = _unpack_out([res.results[c]["outt"] for c in range(NCORES)])
    if trace:
        return out, res
    return out


def kernel(**inputs):
    return run(inputs)



# revision 4
# speedup vs baseline: 1.1586x; 1.1586x over previous
# Trainium2 Bass kernel for nn_BertAdapter_SLT_49933289783411
#
# Reference computation:
#   y   = tt_linear(x) + bias          (TT-factorized 768->768 linear)
#   out = x + gelu_exact(y)
#
# Key math: the TT cores with ranks [1,5,5,5,5,5,1] factor the 768x768
# weight as W = A @ B with A:(768,5), B:(5,768).  We precompute A,B on
# host (tiny, exact) and run a rank-5 bottleneck matmul on device.
#
# Sharding: data-parallel over the batch dim (8 batch elements -> 8 cores).
# Each core handles x_c:(512,768), pre-transposed on host to x^T (feature-
# major) so the contraction dim lands on SBUF partitions.  Per core:
#   t3    = A^T @ x^T              (8,512)   PSUM accumulate over f-chunks
#   y^T_j = B_j^T @ t3_pad         (128,512) per 128-feature output chunk j
#   o^T_j = x^T_j + gelu_exact(y^T_j + bias_j)
# The host transposes the gathered o^T back.
#
# All device I/O is bf16 (packed in pairs into f32 DRAM columns): the
# 2e-2 rel-err budget dwarfs bf16 rounding (~2e-3), and halving the HBM
# bytes halves the DMA-bound portion of the schedule.  Loads are 2 DMAs
# (consts + x half0, x half1), stores 6 (one per 2-chunk group,
# alternating the Sync/Pool DGE queues).  The 512 rows run as two
# pipelined halves so half 0's mm2/gelu/add/store overlaps half 1's
# load; mm1 h1 is emitted after h0's first mm2 so the strict PE FIFO
# doesn't stall h0's output pipeline.
#
# The rank is padded 5->8 so bf16 A-slices stay 4B-aligned.  B_pad row 32
# carries the bias and meets an all-ones row 32 of t3 (gpsimd memset must
# write 32-aligned partition ranges; rows 33..63 of B_pad stay zero).

import numpy as np
import ml_dtypes

import concourse.bass as bass
import concourse.bacc as bacc
import concourse.mybir as mybir
import concourse.tile as tile
from concourse.tile import add_dep_helper
from concourse.bass_utils import run_bass_kernel_spmd

HID = 768
ROWS = 512          # rows per core (one batch element)
HSIZE = (256, 256)
HOFF = (0, 256)
NCORES = 8
FCH = 6             # 768 / 128 feature chunks
RANKP = 8           # TT rank 5 zero-padded to 8 (bf16 alignment)
F32 = mybir.dt.float32
BF16 = mybir.dt.bfloat16

N_WARMUP = 28       # dummy PE matmuls to trip the HAM clock un-throttle

# packed layout of the input tensor, in bf16 columns:
#   [A_pad (128,48)] [B_pad (128,768)] [x h0: c0..c5 x 256] [x h1: ...]
A_COLS = FCH * RANKP                               # 48
BM_COLS = HID                                      # 768
CONST_COLS = A_COLS + BM_COLS                      # 816
XT_COLS = CONST_COLS + FCH * ROWS                  # 3888 bf16 = 1944 f32
OUT_COLS = FCH * ROWS                              # 3072 bf16 = 1536 f32

_CACHE = {}


class _LeanTileContext(tile.TileContext):
    """TileContext with a minimal exit sequence.

    The stock exit emits drain + all-engine barrier + per-sem clears +
    barrier.  The NEFF-level epilogue walrus emits already re-clears the
    whole semaphore space on every execution, so only the drain — which
    makes the kernel end wait for the output DMAs — is kept.
    """

    def _drain_and_barrier(self, tick_clock, wait_clock):
        drain_inst = self.nc.sync.drain()
        wait_clock.add_sem_waits(
            drain_inst.ins, tile.ScopedClock({None: tick_clock.global_clock})
        )
        popped = self.nc._tile_sem_poison_stack.pop()
        assert popped is self._sem_poison


def _xcol(h, c):
    # column (in bf16 units) of x half h, chunk c
    return CONST_COLS + FCH * HOFF[h] + c * HSIZE[h]


def _build_program(act=None):
    if act is None:
        act = mybir.ActivationFunctionType.Gelu
    nc = bacc.Bacc(None, target_bir_lowering=False)
    xt = nc.dram_tensor("xt", [128, XT_COLS // 2], F32, kind="ExternalInput")
    outt = nc.dram_tensor("outt", [128, OUT_COLS // 2], F32, kind="ExternalOutput")

    with _LeanTileContext(nc) as tc:
        with (
            tc.tile_pool(name="const", bufs=1) as cpool,
            tc.tile_pool(name="xs", bufs=1) as xpool,
            tc.tile_pool(name="work", bufs=4) as wpool,
            tc.tile_pool(name="ps_t3", bufs=1, space="PSUM") as tpool,
            tc.tile_pool(name="ps_o", bufs=4, space="PSUM") as opool,
            tc.tile_pool(name="ps_w", bufs=1, space="PSUM") as wps_pool,
        ):
            # --- PE warmup: garbage matmuls so the HAM clock gate opens
            wsb = cpool.tile([128, 128], BF16)
            nc.gpsimd.memset(wsb[:], 0.0)
            wps = wps_pool.tile([128, 128], F32)
            for _ in range(N_WARMUP):
                nc.tensor.matmul(wps[:], wsb[:], wsb[:], start=True, stop=True)

            # t3 in bf16, zero-padded to 128 partitions so mm2 runs K=128;
            # row 32 is all-ones: paired with the bias in B_pad's row 32 it
            # folds the TT bias into mm2.
            t3_sb = cpool.tile([128, ROWS], BF16)
            nc.gpsimd.memset(t3_sb[:], 0.0)
            nc.gpsimd.memset(t3_sb[32:64, :], 1.0)

            x_sb = xpool.tile([128, XT_COLS // 2], F32)
            xb = x_sb[:].bitcast(BF16)                     # (128, XT_COLS)
            a_view = xb[:, 0:A_COLS]                       # (128, 48)
            bm_view = xb[:, A_COLS:CONST_COLS]             # (128, 768)

            t3_ps = [
                tpool.tile([RANKP, HSIZE[h]], F32, name=f"t3_ps{h}") for h in (0, 1)
            ]

            o_sb = xpool.tile([128, OUT_COLS // 2], F32)
            ob = o_sb[:].bitcast(BF16)                     # (128, 3072)

            def dma_load(h):
                # one DMA per half; the first also carries the consts
                s = 0 if h == 0 else _xcol(1, 0) // 2
                e = _xcol(h, FCH - 1) // 2 + HSIZE[h] // 2
                nc.sync.dma_start(x_sb[:, s:e], xt[:, s:e])

            def mm1_half(h, after=None):
                for c in range(FCH):
                    mm = nc.tensor.matmul(
                        t3_ps[h][:],
                        a_view[:, c * RANKP : (c + 1) * RANKP],
                        xb[:, _xcol(h, c) : _xcol(h, c) + HSIZE[h]],
                        start=(c == 0),
                        stop=(c == FCH - 1),
                    )
                    if after is not None:
                        add_dep_helper(
                            mm.ins, after.ins, sync=False,
                            reason="mm1 h1 after phase2 h0 matmuls",
                        )

            def _ocol(h, j):
                return h * FCH * HSIZE[h] + j * HSIZE[h]

            def phase2_half(h):
                sz, off = HSIZE[h], HOFF[h]
                nc.vector.tensor_copy(t3_sb[0:RANKP, off : off + sz], t3_ps[h][:])
                first_mm = None
                for j0 in range(0, FCH, 2):
                    # two output chunks share one PSUM bank
                    o_ps = opool.tile([128, 2 * max(HSIZE)], F32, tag="o_ps")
                    for k in (0, 1):
                        mm = nc.tensor.matmul(
                            o_ps[:, k * sz : (k + 1) * sz],
                            bm_view[:, (j0 + k) * 128 : (j0 + k + 1) * 128],
                            t3_sb[:, off : off + sz],
                            start=(k == 0),
                            stop=(k == 1),
                        )
                        if first_mm is None:
                            first_mm = mm
                    g_sb = wpool.tile([128, 2 * max(HSIZE)], BF16, tag="g_sb", bufs=4)
                    nc.scalar.activation(
                        g_sb[:, : 2 * sz], o_ps[:, : 2 * sz], act, scale=1.0
                    )
                    oslice = ob[:, _ocol(h, j0) : _ocol(h, j0) + 2 * sz]
                    nc.vector.tensor_add(
                        oslice,
                        g_sb[:, : 2 * sz],
                        xb[:, _xcol(h, j0) : _xcol(h, j0) + 2 * sz],
                    )
                    dma = nc.sync if (h * 3 + j0 // 2) % 2 == 0 else nc.gpsimd
                    dma.dma_start(
                        outt[:, _ocol(h, j0) // 2 : (_ocol(h, j0) + 2 * sz) // 2],
                        o_sb[:, _ocol(h, j0) // 2 : (_ocol(h, j0) + 2 * sz) // 2],
                    )
                return first_mm

            dma_load(0)
            dma_load(1)
            mm1_half(0)
            first_mm_h0 = phase2_half(0)
            mm1_half(1, after=first_mm_h0)
            phase2_half(1)

    nc.finalize()
    return nc


def _get_program():
    if "nc" not in _CACHE:
        _CACHE["nc"] = _build_program()
    return _CACHE["nc"]


def _host_prep(hidden_states, bias, cores):
    """Collapse TT cores to rank-5 factors; pack consts + x^T per core."""
    c0, c1, c2, c3, c4, c5 = [c.astype(np.float64) for c in cores]
    A = np.einsum("iv,vjw,wkx->ijkx", c0[0], c1, c2).reshape(HID, 5)
    Bm = np.einsum("xpy,yqz,zr->xpqr", c3, c4, c5[:, :, 0]).reshape(5, HID)

    a_p = np.zeros((128, FCH, RANKP), dtype=ml_dtypes.bfloat16)
    a_p[:, :, :5] = A.reshape(FCH, 128, 5).transpose(1, 0, 2)
    a_p = a_p.reshape(128, A_COLS)
    bm_pad = np.zeros((128, HID), dtype=ml_dtypes.bfloat16)
    bm_pad[:5] = Bm.astype(ml_dtypes.bfloat16)
    # row 32 carries the TT bias; it meets the all-ones row 32 of t3_sb in mm2
    bm_pad[32] = bias.astype(ml_dtypes.bfloat16)

    const_block = np.concatenate([a_p, bm_pad], axis=1)   # (128, 816) bf16

    xts = []
    for c in range(NCORES):
        xct = hidden_states[c].T.astype(ml_dtypes.bfloat16)   # (768, 512)
        # [p, h*1536 + c*256 + m~] = x^T[c*128+p, h*256+m~]
        xr = (
            xct.reshape(FCH, 128, 2, HSIZE[0])
            .transpose(1, 2, 0, 3)
            .reshape(128, 2 * FCH * HSIZE[0])
        )
        packed = np.concatenate([const_block, xr], axis=1)    # (128, 3888) bf16
        xts.append(np.ascontiguousarray(packed).view(np.float32))
    return xts


def _unpack_out(outt_list):
    """outt[p, h*1536 + j*256 + m~] = out[h*256+m~, j*128+p] -> (8,512,768)."""
    outs = []
    for outt in outt_list:
        ob = np.ascontiguousarray(outt).view(ml_dtypes.bfloat16)
        o = (
            ob.reshape(128, 2, FCH, HSIZE[0])
            .transpose(1, 3, 2, 0)
            .reshape(ROWS, HID)
        )
        outs.append(o.astype(np.float32))
    return np.stack(outs, axis=0)


def run(inputs, trace=False, **spmd_kwargs):
    hidden_states = np.asarray(inputs["hidden_states"], dtype=np.float32)
    bias = np.asarray(inputs["bias"], dtype=np.float32)
    cores = [np.asarray(inputs[f"core{i}"], dtype=np.float32) for i in range(6)]

    xts = _host_prep(hidden_states, bias, cores)
    nc = _get_program()
    in_maps = [{"xt": xts[c]} for c in range(NCORES)]
    res = run_bass_kernel_spmd(
        nc, in_maps, core_ids=list(range(NCORES)), trace=trace, **spmd_kwargs
    )
    out = _unpack_out([res.results[c]["outt"] for c in range(NCORES)])
    if trace:
        return out, res
    return out


def kernel(**inputs):
    return run(inputs)
